# revision 17
# baseline (speedup 1.0000x reference)
"""DenseGAT layer Bass kernel for Trainium2, 8 NeuronCores.

Sharding: core c handles batch b=c//2, row-half r=c%2 (1024 dest rows).

Algorithm (per core), avoiding any per-element transcendental over the NxN
score matrix:
  s[j,i] = as_i + ad_j;  E = exp(leaky_relu(s)) = max(g_i*h_j, u_i*v_j)
  with g=exp(as), h=exp(ad), u=exp(0.2 as), v=exp(0.2 ad).
  With C = [s>=0] and mask m (adj^T + I, 0/1):  p = m*C*g*h + m*(1-C)*u*v
  Normalized by g_i (softmax is scale-invariant per row):
    num'[o,i] = (WhH^T @ mC)[o,i] + w_i * (WhV^T @ (m - mC))[o,i]
  where WhH = Wh*h_j, WhV = Wh*v_j (plus h/v column for the denominator),
  w_i = exp(-0.8 as_i).  out = relu(num'/denom').
Heads 0..NCMP-1 ("cmp-heads"): mC = is_ge(as_bcast, -ad) * m on DVE; the
m-part is a shared head-packed PE stream (Rm) combined after transpose.
Heads NCMP..7 ("sign-heads"): T = sign(s) * m with Sign on ACT; mC = (m+T)/2
is realized by accumulating a per-head m-stream and T-stream (h,v halved via
exp(x - ln2)) into the same PSUM region; single-STT combine, no Rm share.
"""
import sys
import numpy as np

for _p in ("/opt/trn_rl_repo", "/root/.axon_site/_ro/trn_rl_repo"):
    if _p not in sys.path:
        sys.path.insert(0, _p)

B, N, F, O, H = 4, 2048, 128, 32, 8
NI = 1024          # dest rows per core
NJB = N // 128     # 16 j-blocks
NIT = NI // 128    # 8 i-tiles
NCMP = 5           # heads 0..NCMP-1 use the DVE compare path; rest sign path
LN2 = 0.6931471805599453

_CACHE = {}


def build_nc():
    import concourse.bacc as bacc
    import concourse.tile as tile
    import concourse.mybir as mybir

    f32 = mybir.dt.float32
    f32r = mybir.dt.float32r
    bf = mybir.dt.bfloat16
    AF = mybir.ActivationFunctionType
    OP = mybir.AluOpType

    nc = bacc.Bacc(None, target_bir_lowering=False)
    xcat_d = nc.dram_tensor("xcat", [N + NI, F], f32, kind="ExternalInput")
    adjs_d = nc.dram_tensor("adjs", [NI, N], f32, kind="ExternalInput")
    wt_d = nc.dram_tensor("wt", [F, H * O], f32, kind="ExternalInput")
    a2t4_d = nc.dram_tensor("a2t4", [F, 2 * H], f32, kind="ExternalInput")
    y_d = nc.dram_tensor("y", [NI, H * O], f32, kind="ExternalOutput")

    # Rm packing for cmp-heads: groups of <=3 heads
    rm_groups = []
    h0 = 0
    while h0 < NCMP:
        ng = min(3, NCMP - h0)
        rm_groups.append((h0, ng))
        h0 += ng
    rm_off = {}
    off = 0
    for hh0, ng in rm_groups:
        for k in range(ng):
            rm_off[hh0 + k] = off + 33 * k
        off += 33 * ng
    rm_w = off  # total packed columns

    with tile.TileContext(nc) as tc:
      with tc.tile_pool(name="persist", bufs=1) as pp:
        with (
            tc.tile_pool(name="ld", bufs=3) as ld,
            tc.tile_pool(name="tp", bufs=4, space="PSUM") as tps,
        ):
            # ---- constants ----
            ones = pp.tile([128, 128], f32, tag="ones")
            nc.vector.memset(ones[:], 1.0)
            tri = pp.tile([128, 128], f32, tag="tri")
            ident = pp.tile([128, 128], f32, tag="ident")
            nc.gpsimd.affine_select(tri[:], ones[:], [[-1, 128]], OP.is_ge, 0.0,
                                    base=0, channel_multiplier=1)
            nc.gpsimd.affine_select(ident[:], tri[:], [[1, 128]], OP.is_ge, 0.0,
                                    base=0, channel_multiplier=-1)
            identb = pp.tile([128, 128], bf, tag="identb")
            nc.vector.tensor_copy(identb[:], ident[:])
            sel = pp.tile([16, H * 128], f32, tag="sel")
            nc.vector.memset(sel[:], 1.0)
            nc.gpsimd.affine_select(sel[:], sel[:], [[-2, 8], [0, 128]],
                                    OP.is_equal, 0.0, base=0,
                                    channel_multiplier=1)

            # ---- adj^T (mask) mT [128, NJB, NI] bf16 (DMA issued early) ----
            mT = pp.tile([128, NJB, NI], bf, tag="mT")
            for it in range(NIT):
                adjn = ld.tile([128, N], f32, tag="adjn")
                nc.sync.dma_start(adjn[:], adjs_d[it * 128:(it + 1) * 128, :])
                # adj is 0/1 valued: high half of each f32 is the exact bf16
                adjhi = adjn[:].bitcast(bf)[:, 1::2]
                for k4 in range(4):
                    aj_p = tps.tile([128, 4, 128], bf, tag="ps")
                    for q in range(4):
                        jb = 4 * k4 + q
                        nc.tensor.transpose(
                            aj_p[:, q, :], adjhi[:, jb * 128:(jb + 1) * 128],
                            identb[:])
                    nc.scalar.copy(
                        mT[:, 4 * k4:4 * k4 + 4, it * 128:(it + 1) * 128], aj_p[:])

            # ---- xT [F, 3072] ----
            xT = pp.tile([F, N + NI], f32, tag="xT")
            for t in range(24):
                xt_l = ld.tile([128, F], f32, tag="xload")
                nc.sync.dma_start(xt_l[:], xcat_d[t * 128:(t + 1) * 128, :])
                xt_p = tps.tile([128, 128], f32, tag="ps")
                nc.tensor.transpose(xt_p[:], xt_l[:], ident[:])
                nc.any.tensor_copy(xT[:, t * 128:(t + 1) * 128], xt_p[:])

            # ---- W / a projections ----
            wtsb = pp.tile([F, H * O], f32, tag="wtsb")
            nc.sync.dma_start(wtsb[:], wt_d[:])
            a2sb = pp.tile([F, 2 * H], f32, tag="a2sb")
            nc.sync.dma_start(a2sb[:], a2t4_d[:])
            wt0 = pp.tile([128, 128], f32, tag="wt0")
            wt1 = pp.tile([128, 128], f32, tag="wt1")
            for half, dst in ((0, wt0), (1, wt1)):
                w_p = tps.tile([128, 128], f32, tag="ps")
                nc.tensor.transpose(w_p[:], wtsb[:, half * 128:(half + 1) * 128],
                                    ident[:])
                nc.any.tensor_copy(dst[:], w_p[:])
            # atil rows: 2h = W_h@a_src_h, 2h+1 = W_h@a_dst_h (block-diag a2sb)
            asdT = pp.tile([F, 16], f32, tag="asdT")
            for half, wthalf in ((0, wt0), (1, wt1)):
                ap_p = tps.tile([8, 128], f32, tag="ps")
                nc.tensor.matmul(ap_p[:], a2sb[:, half * 8:(half + 1) * 8],
                                 wthalf[:])
                asdh = ld.tile([8, F], f32, tag="asdh")
                nc.any.tensor_copy(asdh[:], ap_p[:])
                at2_p = tps.tile([128, 8], f32, tag="ps")
                nc.tensor.transpose(at2_p[:], asdh[:], ident[0:8, 0:8])
                nc.any.tensor_copy(asdT[:, half * 8:(half + 1) * 8], at2_p[:])

            # ---- alphas [16, 3072]: row 2h = as_h, 2h+1 = ad_h ----
            alph = pp.tile([16, N + NI], f32, tag="alph")
            for cch in range(6):
                al_p = tps.tile([16, 512], f32, tag="ps")
                nc.tensor.matmul(al_p[:], asdT[:],
                                 xT[:, cch * 512:(cch + 1) * 512])
                nc.any.tensor_copy(alph[:, cch * 512:(cch + 1) * 512], al_p[:])
            # alphT [128, 24, 16]
            alphT = pp.tile([128, 24, 16], f32, tag="alphT")
            for t in range(24):
                at_p = tps.tile([128, 16], f32, tag="ps")
                nc.tensor.transpose(at_p[:], alph[:, t * 128:(t + 1) * 128],
                                    ident[0:16, 0:16])
                nc.any.tensor_copy(alphT[:, t, :], at_p[:])

            # ---- per-node exponential columns ----
            negad = pp.tile([128, NJB, NCMP], f32, tag="negad")
            hcol = pp.tile([128, NJB, H], f32, tag="hcol")
            vneg = pp.tile([128, NJB, H], f32, tag="vneg")
            vpos = pp.tile([128, NJB, H - NCMP], f32, tag="vpos")
            negw = pp.tile([128, NIT, H], f32, tag="negw")
            bln2 = pp.tile([128, 1], f32, tag="bln2")
            nc.vector.memset(bln2[:], -LN2)
            adc_c = alphT[:, 0:NJB, 1:2 * NCMP:2]          # cmp-head ad cols
            adc_s = alphT[:, 0:NJB, 1 + 2 * NCMP::2]       # sign-head ad cols
            nc.vector.tensor_scalar_mul(negad[:], adc_c, -1.0)
            # h = exp(ad) (cmp) / 0.5*exp(ad) (sign); v likewise at 0.2 scale
            nc.scalar.activation(hcol[:, :, 0:NCMP], adc_c, AF.Exp)
            nc.scalar.activation(hcol[:, :, NCMP:H], adc_s, AF.Exp, bias=bln2[:])
            nc.scalar.activation(vneg[:, :, 0:NCMP], adc_c, AF.Exp, scale=0.2)
            nc.scalar.activation(vneg[:, :, NCMP:H], adc_s, AF.Exp, scale=0.2,
                                 bias=bln2[:])
            nc.vector.tensor_copy(vpos[:], vneg[:, :, NCMP:H])
            nc.vector.tensor_scalar_mul(vneg[:], vneg[:], -1.0)
            # vpos now = +0.5*exp(0.2 ad) for sign heads; vneg = -(h/v scale)
            ascols_i = alphT[:, NJB:24, 0::2]
            nc.scalar.activation(negw[:], ascols_i, AF.Exp, scale=-0.8)
            nc.vector.tensor_scalar_mul(negw[:], negw[:], -1.0)

            # ---- as broadcast tiles [128, H, NI] bf16 ----
            asb = pp.tile([128, H, NI], bf, tag="asb")
            for h in range(H):
                for cch in range(2):
                    ab_p = tps.tile([128, 512], f32, tag="ps")
                    nc.tensor.matmul(
                        ab_p[:], sel[:, h * 128:(h + 1) * 128],
                        alph[:, N + cch * 512:N + (cch + 1) * 512])
                    nc.scalar.copy(
                        asb[:, h, cch * 512:(cch + 1) * 512], ab_p[:])

            # ---- Wh and scaled weight tiles ----
            # whHV[..,h,0:33]=[Wh*h | h]; [..,h,33:66]=[-Wh*v | -v]
            # whSm (sign-head m-stream lhsT): [Wh*h | h | +Wh*v | +v] (halved)
            whHV = pp.tile([128, NJB, H, 66], bf, tag="whHV")
            whVm = pp.tile([128, NJB, NCMP, 33], bf, tag="whVm")
            whSm = pp.tile([128, NJB, H - NCMP, 66], bf, tag="whSm")
            for jb in range(NJB):
                wh_p = tps.tile([128, H * O], f32, tag="ps")
                nc.tensor.matmul(
                    wh_p[:], xT[:, jb * 128:(jb + 1) * 128], wtsb[:])
                whv = wh_p[:].rearrange("p (h o) -> p h o", h=H)
                whv_s = whv[:, NCMP:H, :]
                hb = hcol[:, jb, :].unsqueeze(2).broadcast_to([128, H, O])
                vb = vneg[:, jb, :].unsqueeze(2).broadcast_to([128, H, O])
                vpb = vpos[:, jb, :].unsqueeze(2).broadcast_to(
                    [128, H - NCMP, O])
                nc.vector.tensor_mul(whHV[:, jb, :, 0:32], whv, hb)
                nc.vector.tensor_mul(whHV[:, jb, :, 33:65], whv, vb)
                nc.vector.tensor_copy(whHV[:, jb, :, 32:33],
                                      hcol[:, jb, :].unsqueeze(2))
                nc.vector.tensor_copy(whHV[:, jb, :, 65:66],
                                      vneg[:, jb, :].unsqueeze(2))
                nc.vector.tensor_copy(whSm[:, jb, :, 0:33],
                                      whHV[:, jb, NCMP:H, 0:33])
                nc.vector.tensor_mul(whSm[:, jb, :, 33:65], whv_s, vpb)
                nc.vector.tensor_copy(whSm[:, jb, :, 65:66],
                                      vpos[:, jb, :].unsqueeze(2))
                nc.vector.tensor_copy(whVm[:, jb, :, :],
                                      whHV[:, jb, 0:NCMP, 33:66])

            # ---- Rm: head-packed (-WhV_ext)^T @ m for cmp-heads ----
            rm_sb = pp.tile([99, len(rm_groups), NI], f32, tag="rmsb")
            with tc.tile_pool(name="rmps", bufs=2, space="PSUM") as rmps:
                for g, (hh0, ng) in enumerate(rm_groups):
                    rm_p = rmps.tile([99, NI], f32, tag="rmp")
                    for jb in range(NJB):
                        lhs = whVm[:, jb, hh0:hh0 + ng, :]
                        for cch in range(2):
                            nc.tensor.matmul(
                                rm_p[0:33 * ng, cch * 512:(cch + 1) * 512],
                                lhs, mT[:, jb, cch * 512:(cch + 1) * 512],
                                start=(jb == 0), stop=(jb == NJB - 1))
                    nc.scalar.copy(rm_sb[0:33 * ng, g, :], rm_p[0:33 * ng, :])
            # RmT [128, NIT, rm_w] f32
            rmt = pp.tile([128, NIT, rm_w], f32, tag="rmt")
            for it in range(NIT):
                rt_p = tps.tile([128, rm_w], f32, tag="ps")
                coff = 0
                for g, (hh0, ng) in enumerate(rm_groups):
                    nc.tensor.transpose(
                        rt_p[:, coff:coff + 33 * ng],
                        rm_sb[0:33 * ng, g, it * 128:(it + 1) * 128],
                        ident[0:33 * ng, 0:33 * ng])
                    coff += 33 * ng
                nc.any.tensor_copy(rmt[:, it, :], rt_p[:])

        # ---- main head loop ----
        osb = pp.tile([128, NIT, H * O], f32, tag="osb")
        with (
            tc.tile_pool(name="mc", bufs=4) as mcp,
            tc.tile_pool(name="rps", bufs=2, space="PSUM") as rps,
            tc.tile_pool(name="rtps", bufs=3, space="PSUM") as rtps,
            tc.tile_pool(name="cmb", bufs=4) as cmb,
            tc.tile_pool(name="small", bufs=2) as sm,
        ):
            head_order = [0, 5, 1, 6, 2, 7, 3, 4][:H]
            for h in head_order:
                r_p = rps.tile([66, NI], f32, tag="rp")
                if h < NCMP:
                    for jb in range(NJB):
                        cmp_t = mcp.tile([128, NI], bf, tag="cmp")
                        nc.vector.tensor_scalar(
                            cmp_t[:], asb[:, h, :], negad[:, jb, h:h + 1],
                            None, OP.is_ge)
                        mc = mcp.tile([128, NI], bf, tag="mc")
                        nc.vector.tensor_tensor(mc[:], cmp_t[:], mT[:, jb, :],
                                                OP.mult)
                        for cch in range(2):
                            nc.tensor.matmul(
                                r_p[:, cch * 512:(cch + 1) * 512],
                                whHV[:, jb, h, :],
                                mc[:, cch * 512:(cch + 1) * 512],
                                start=(jb == 0), stop=(jb == NJB - 1))
                else:
                    hs = h - NCMP
                    for jb in range(NJB):
                        sg = mcp.tile([128, NI], bf, tag="cmp")
                        nc.scalar.activation(
                            sg[:], asb[:, h, :], AF.Sign,
                            bias=alphT[:, jb, 2 * h + 1:2 * h + 2])
                        mc = mcp.tile([128, NI], bf, tag="mc")
                        nc.vector.tensor_tensor(mc[:], sg[:], mT[:, jb, :],
                                                OP.mult)
                        for cch in range(2):
                            nc.tensor.matmul(
                                r_p[:, cch * 512:(cch + 1) * 512],
                                whSm[:, jb, hs, :],
                                mT[:, jb, cch * 512:(cch + 1) * 512],
                                start=(jb == 0), stop=False)
                            nc.tensor.matmul(
                                r_p[:, cch * 512:(cch + 1) * 512],
                                whHV[:, jb, h, :],
                                mc[:, cch * 512:(cch + 1) * 512],
                                start=False, stop=(jb == NJB - 1))
                rsb = sm.tile([66, NI], f32, tag="rsb")
                nc.scalar.copy(rsb[:], r_p[:])
                for it in range(NIT):
                    rt_ps = rtps.tile([128, 66], f32, tag="rtp")
                    nc.tensor.transpose(
                        rt_ps[:], rsb[:, it * 128:(it + 1) * 128],
                        ident[0:66, 0:66])
                    nn = cmb.tile([128, 33], f32, tag="nn")
                    if h < NCMP:
                        q = cmb.tile([128, 33], f32, tag="q")
                        nc.vector.scalar_tensor_tensor(
                            q[:], rmt[:, it, rm_off[h]:rm_off[h] + 33],
                            negw[:, it, h:h + 1], rt_ps[:, 0:33],
                            OP.mult, OP.add)
                        nc.vector.scalar_tensor_tensor(
                            nn[:], rt_ps[:, 33:66], negw[:, it, h:h + 1],
                            q[:], OP.mult, OP.subtract)
                    else:
                        r1s = cmb.tile([128, 33], f32, tag="q")
                        nc.vector.tensor_copy(r1s[:], rt_ps[:, 0:33])
                        nc.vector.scalar_tensor_tensor(
                            nn[:], rt_ps[:, 33:66], negw[:, it, h:h + 1],
                            r1s[:], OP.mult, OP.subtract)
                    rc = cmb.tile([128, 1], f32, tag="rc")
                    nc.vector.reciprocal(rc[:], nn[:, 32:33])
                    nc.scalar.activation(
                        osb[:, it, h * O:(h + 1) * O], nn[:, 0:32],
                        AF.Relu, scale=rc[:])
        for it in range(NIT):
            nc.sync.dma_start(y_d[it * 128:(it + 1) * 128, :], osb[:, it, :])

    nc.compile()
    return nc


def shard_inputs(x, adj, W, a_src, a_dst):
    """Host-side marshalling (layout only) for each of the 8 cores."""
    in_maps = []
    # block-diagonal a layout: a2t4[32q+o, 8*half + 2q + d] = a_{d}_{4*half+q}[o]
    a2t4 = np.zeros((128, 16), np.float32)
    for h in range(H):
        half, q = h // 4, h % 4
        a2t4[32 * q:32 * q + 32, 8 * half + 2 * q] = a_src[h]
        a2t4[32 * q:32 * q + 32, 8 * half + 2 * q + 1] = a_dst[h]
    wt = np.ascontiguousarray(W.transpose(1, 0, 2).reshape(F, H * O))
    for c in range(8):
        b, r = c // 2, c % 2
        rows = slice(r * NI, (r + 1) * NI)
        xcat = np.concatenate([x[b], x[b][rows]], axis=0)
        adjs = np.ascontiguousarray(adj[b][rows])
        idx = np.arange(NI)
        adjs[idx, r * NI + idx] = 1.0
        in_maps.append({
            "xcat": np.ascontiguousarray(xcat, np.float32),
            "adjs": adjs.astype(np.float32, copy=False),
            "wt": np.ascontiguousarray(wt, np.float32),
            "a2t4": np.ascontiguousarray(a2t4, np.float32),
        })
    return in_maps


def kernel(x, adj, W, a_src, a_dst, _trace=False):
    from concourse.bass_utils import run_bass_kernel_spmd

    if "nc" not in _CACHE:
        _CACHE["nc"] = build_nc()
    nc = _CACHE["nc"]
    x = np.asarray(x, np.float32)
    adj = np.asarray(adj, np.float32)
    W = np.asarray(W, np.float32)
    a_src = np.asarray(a_src, np.float32)
    a_dst = np.asarray(a_dst, np.float32)
    in_maps = shard_inputs(x, adj, W, a_src, a_dst)
    res = run_bass_kernel_spmd(nc, in_maps, core_ids=list(range(8)),
                               trace=_trace)
    out = np.zeros((B, N, H * O), np.float32)
    for c in range(8):
        b, r = c // 2, c % 2
        out[b, r * NI:(r + 1) * NI, :] = res.results[c]["y"]
    _CACHE["last_result"] = res
    return out
